# revision 2
# baseline (speedup 1.0000x reference)
"""Single-head attention on 8 Trainium2 NeuronCores.

Problem: x[8, 2048, 768], Wq/Wk/Wv[768, 64]+biases, mask[2048, 2048] int32
Output:  softmax(mask(Q K^T / 8)) V   -> [8, 2048, 64] f32

Sharding: data-parallel over batch — core b computes batch element b.

Per-core dataflow (all matmuls bf16 in / fp32 psum accumulate):
  host:  xT = x[b].T (w-major), Wqk = [Wq | Wk/8], mT = mask.T (k-major, 0/1)
  QK:    QK[n,128] = xT.T @ Wqk + bqk  (psum), cast bf16 -> QQ/KK duplicated
         across partition halves (for row-tiled score matmuls)
  V:     V[n,64] = xT.T @ Wv + bv, stored as V'[k,65] with ones column so the
         PV matmul also produces the softmax denominator for free
  ST:    ST[k,q] = KK.T @ QQ per 128-k-chunk (K=64 contraction: two chunks run
         concurrently in the PE array via row tiling at partitions 0/64)
  P:     P = exp(ST) on ScalarE (psum -> sbuf bf16), P *= mT (VectorE)
  OT:    OT[65,q] += V'[kchunk].T @ P[kchunk]  (accumulate over 16 k-chunks)
  out:   transpose OT 128-col blocks on PE, out[q,64] = OT[q,:64]/OT[q,64]
"""

import numpy as np
import ml_dtypes

import bass_rust
import concourse.bass as bass
import concourse.mybir as mybir
import concourse.tile as tile
from concourse.bass_utils import run_bass_kernel_spmd

BF16 = ml_dtypes.bfloat16
F32 = mybir.dt.float32
BF = mybir.dt.bfloat16

N_CORES = 8
SEQ = 2048
WIDTH = 768
HEAD = 64
NCH = WIDTH // 128      # 6 contraction chunks for the projections
NKC = SEQ // 128        # 16 key chunks
QT = 1024               # q tile (columns processed per main-loop sweep)
NQT = SEQ // QT


def _split_excess_waits(nc, max_waits=1):
    """walrus in this container rejects >1 sync wait per instruction; hoist
    extras onto preceding same-engine NoOps (same semantics: the engine
    executes its stream in order, so waiting earlier is equivalent)."""
    n = 0
    for bb in nc.main_func.blocks:
        new_list = []
        for ins in bb.instructions:
            si = ins.sync_info
            if si is not None and len(si.on_wait) > max_waits:
                waits = list(si.on_wait)
                extra, keep = waits[:-max_waits], waits[-max_waits:]
                for j, w in enumerate(extra):
                    nop = bass_rust.InstNoOp(
                        name=f"{ins.name}-ws{j}", engine=ins.engine, ins=[], outs=[]
                    )
                    nop.sync_info = mybir.SyncInfo(on_wait=[w], on_update=[])
                    new_list.append(nop)
                    n += 1
                ins.sync_info = mybir.SyncInfo(
                    on_wait=keep, on_update=list(si.on_update)
                )
            new_list.append(ins)
        bb.instructions = new_list
    return n


def _build():
    nc = bass.Bass("TRN2", target_bir_lowering=False, debug=False,
                   num_devices=N_CORES)

    xT_d = nc.declare_dram_parameter("xT", [WIDTH, SEQ], BF, isOutput=False).ap()
    wqk_d = nc.declare_dram_parameter("Wqk", [WIDTH, 128], BF, isOutput=False).ap()
    bqk_d = nc.declare_dram_parameter("bqk", [128, 1], F32, isOutput=False).ap()
    wv_d = nc.declare_dram_parameter("Wv", [WIDTH, HEAD], BF, isOutput=False).ap()
    bv_d = nc.declare_dram_parameter("bv", [1, HEAD], BF, isOutput=False).ap()
    mT_d = nc.declare_dram_parameter("mT", [SEQ, SEQ], BF, isOutput=False).ap()
    i65_d = nc.declare_dram_parameter("I65", [HEAD + 1, HEAD + 1], F32,
                                      isOutput=False).ap()
    out_d = nc.declare_dram_parameter("out", [SEQ, HEAD], F32, isOutput=True).ap()

    ADD = mybir.AluOpType.add
    MUL = mybir.AluOpType.mult
    EXP = mybir.ActivationFunctionType.Exp

    with tile.TileContext(nc) as tc:
        with (
            tc.tile_pool(name="const", bufs=1) as const,
            tc.tile_pool(name="pp", bufs=4) as ppool,
            tc.tile_pool(name="ep", bufs=2) as epool,
            tc.tile_pool(name="op", bufs=4) as opool,
            tc.tile_pool(name="rp", bufs=4) as rpool,
            tc.tile_pool(name="stp", bufs=2, space="PSUM") as stp,
            tc.tile_pool(name="otp", bufs=1, space="PSUM") as otp,
            tc.tile_pool(name="scp", bufs=2, space="PSUM") as scp,
        ):
            # ---- constants / inputs into SBUF ----
            xt = const.tile([128, NCH, SEQ], BF)
            xt_src = xT_d.rearrange("(c p) q -> p c q", p=128)
            for c in range(NCH):
                nc.sync.dma_start(out=xt[:, c, :], in_=xt_src[:, c, :])

            wqk = const.tile([128, NCH, 128], BF)
            nc.sync.dma_start(out=wqk, in_=wqk_d.rearrange("(c p) n -> p c n", p=128))
            wv = const.tile([128, NCH, HEAD], BF)
            nc.sync.dma_start(out=wv, in_=wv_d.rearrange("(c p) n -> p c n", p=128))
            bqk = const.tile([128, 1], F32)
            nc.sync.dma_start(out=bqk, in_=bqk_d)
            bv = const.tile([1, HEAD], BF)
            nc.sync.dma_start(out=bv, in_=bv_d)
            i65 = const.tile([HEAD + 1, HEAD + 1], F32)
            nc.sync.dma_start(out=i65, in_=i65_d)
            ones = const.tile([1, 128], BF)
            nc.vector.memset(ones, 1.0)

            mt = const.tile([128, NKC, SEQ], BF)
            mt_src = mT_d.rearrange("(c p) q -> p c q", p=128)
            for c in range(NKC):
                nc.sync.dma_start(out=mt[:, c, :], in_=mt_src[:, c, :])

            # ---- projections ----
            qktmp = const.tile([128, SEQ], BF)   # Q on parts 0:64, K on 64:128
            for g in range(SEQ // 1024):
                qk_ps = stp.tile([128, 1024], F32, tag="st")
                for t in range(2):
                    cols = slice(g * 1024 + t * 512, g * 1024 + (t + 1) * 512)
                    for c in range(NCH):
                        nc.tensor.matmul(
                            qk_ps[:, t * 512:(t + 1) * 512],
                            wqk[:, c, :], xt[:, c, cols],
                            start=(c == 0), stop=(c == NCH - 1),
                        )
                nc.vector.tensor_scalar(
                    out=qktmp[:, g * 1024:(g + 1) * 1024], in0=qk_ps,
                    scalar1=bqk[:, 0:1], scalar2=None, op0=ADD,
                )

            qq = const.tile([128, SEQ], BF)      # Q duplicated on both halves
            kk = const.tile([128, SEQ], BF)      # K duplicated on both halves
            nc.vector.tensor_copy(out=qq[0:64, :], in_=qktmp[0:64, :])
            nc.vector.tensor_copy(out=qq[64:128, :], in_=qktmp[0:64, :])
            nc.vector.tensor_copy(out=kk[0:64, :], in_=qktmp[64:128, :])
            nc.vector.tensor_copy(out=kk[64:128, :], in_=qktmp[64:128, :])

            vp = const.tile([128, NKC, HEAD + 1], BF)   # V' with ones column
            for i in range(4):
                v_ps = scp.tile([128, 4, HEAD], F32, tag="sc")
                for j in range(4):
                    s = 4 * i + j
                    for c in range(NCH):
                        nc.tensor.matmul(
                            v_ps[:, j, :], xt[:, c, s * 128:(s + 1) * 128],
                            wv[:, c, :], start=(c == 0), stop=False,
                        )
                    nc.tensor.matmul(
                        v_ps[:, j, :], ones[0:1, :], bv[0:1, :],
                        start=False, stop=True,
                    )
                nc.vector.tensor_copy(
                    out=vp[:, 4 * i:4 * (i + 1), 0:HEAD], in_=v_ps
                )
            nc.vector.memset(vp[:, :, HEAD:HEAD + 1], 1.0)

            # ---- main loop: scores -> exp -> mask -> PV ----
            for q in range(NQT):
                qc = slice(q * QT, (q + 1) * QT)
                ot_ps = otp.tile([HEAD + 1, QT], F32)
                for kp in range(NKC // 2):
                    k0, k1 = 2 * kp, 2 * kp + 1
                    st_a = stp.tile([128, QT], F32, tag="st")
                    st_b = stp.tile([128, QT], F32, tag="st")
                    for h in range(2):
                        qh = slice(q * QT + h * 512, q * QT + (h + 1) * 512)
                        nc.tensor.matmul(
                            st_a[:, h * 512:(h + 1) * 512],
                            kk[0:64, k0 * 128:(k0 + 1) * 128], qq[0:64, qh],
                            start=True, stop=True,
                        )
                        nc.tensor.matmul(
                            st_b[:, h * 512:(h + 1) * 512],
                            kk[64:128, k1 * 128:(k1 + 1) * 128], qq[64:128, qh],
                            start=True, stop=True,
                        )
                    p_a = ppool.tile([128, QT], BF, tag="p")
                    p_b = ppool.tile([128, QT], BF, tag="p")
                    nc.scalar.activation(p_a, st_a, EXP)
                    nc.scalar.activation(p_b, st_b, EXP)
                    nc.vector.tensor_mul(p_a, p_a, mt[:, k0, qc])
                    nc.vector.tensor_mul(p_b, p_b, mt[:, k1, qc])
                    for h in range(2):
                        hs = slice(h * 512, (h + 1) * 512)
                        nc.tensor.matmul(
                            ot_ps[:, hs], vp[:, k0, :], p_a[:, hs],
                            start=(kp == 0), stop=False,
                        )
                        nc.tensor.matmul(
                            ot_ps[:, hs], vp[:, k1, :], p_b[:, hs],
                            start=False, stop=(kp == NKC // 2 - 1),
                        )

                # ---- epilogue: transpose, normalize, store ----
                ot_sb = epool.tile([HEAD + 1, QT], F32)
                nc.vector.tensor_copy(out=ot_sb, in_=ot_ps)
                for s in range(QT // 128):
                    otr = scp.tile([128, 4, HEAD], F32, tag="sc")
                    otr65 = otr.rearrange("p a b -> p (a b)")[:, 0:HEAD + 1]
                    nc.tensor.transpose(
                        otr65, ot_sb[:, s * 128:(s + 1) * 128], i65
                    )
                    rec = rpool.tile([128, 1], F32)
                    nc.vector.reciprocal(rec, otr65[:, HEAD:HEAD + 1])
                    o_sb = opool.tile([128, HEAD], F32)
                    nc.vector.tensor_scalar(
                        out=o_sb, in0=otr65[:, 0:HEAD],
                        scalar1=rec[:, 0:1], scalar2=None, op0=MUL,
                    )
                    row = (q * (QT // 128) + s) * 128
                    nc.sync.dma_start(out=out_d[row:row + 128, :], in_=o_sb)

    _split_excess_waits(nc)
    return nc


_CACHE = {}


def _get_nc():
    if "nc" not in _CACHE:
        _CACHE["nc"] = _build()
    return _CACHE["nc"]


def _prep_in_maps(x, Wq, bq, Wk, bk, Wv, bv, mask):
    x = np.asarray(x, dtype=np.float32)
    Wqk = np.concatenate(
        [np.asarray(Wq, np.float32), np.asarray(Wk, np.float32) * 0.125], axis=1
    ).astype(BF16)
    bqk = np.concatenate(
        [np.asarray(bq, np.float32), np.asarray(bk, np.float32) * 0.125]
    ).astype(np.float32).reshape(128, 1)
    Wv16 = np.asarray(Wv, np.float32).astype(BF16)
    bv16 = np.asarray(bv, np.float32).astype(BF16).reshape(1, HEAD)
    mT = np.ascontiguousarray(np.asarray(mask).T).astype(BF16)
    i65 = np.eye(HEAD + 1, dtype=np.float32)
    in_maps = []
    for b in range(N_CORES):
        xT = np.ascontiguousarray(x[b].T).astype(BF16)
        in_maps.append({
            "xT": xT, "Wqk": Wqk, "bqk": bqk, "Wv": Wv16, "bv": bv16,
            "mT": mT, "I65": i65,
        })
    return in_maps


def _run(in_maps, trace=False, **kw):
    nc = _get_nc()
    return run_bass_kernel_spmd(nc, in_maps, list(range(N_CORES)), trace=trace, **kw)


def kernel(x, Wq, bq, Wk, bk, Wv, bv, mask):
    in_maps = _prep_in_maps(x, Wq, bq, Wk, bk, Wv, bv, mask)
    res = _run(in_maps)
    return np.stack([np.asarray(res.results[b]["out"]) for b in range(N_CORES)])
